# revision 1
# baseline (speedup 1.0000x reference)
"""ConstituentAttention Trainium2 kernel.

Math (derived from the reference):
  - score is masked to the super/sub-diagonal only, so the row softmax is a
    2-element softmax over s_u[i] = q_i.k_{i+1}/E and s_l[i] = q_i.k_{i-1}/E.
  - neighbor_attn = prior + (1-prior)*g where g == sqrt(1e-9) =: C0 everywhere
    except g[i,i+1] = g[i+1,i] = sqrt(a_u[i]*a_l[i+1] + 1e-9) =: g_u[i].
  - log-space prefix products collapse to c_attn[i,j] = exp(-|U[j]-U[i]|) for
    i != j, where U = exclusive prefix sum of u_i = log(na[i,i+1] + 1e-9);
    diagonal of c_attn = na[i,i].

Sharding: data-parallel over batch, one batch element per NeuronCore (B=8).
"""

import numpy as np

import concourse.bass as bass
import concourse.tile as tile
from concourse import mybir
from concourse.bass_utils import run_bass_kernel_spmd

S, B, E, P = 1024, 8, 512, 64
P2 = 2 * P
NB = S // 128
C0 = float(np.sqrt(1e-9))
F32 = mybir.dt.float32
AF = mybir.ActivationFunctionType
ALU = mybir.AluOpType

_CACHE = {}


def _ap(handle_or_ap, offset, dims):
    a0 = handle_or_ap[:] if not isinstance(handle_or_ap, bass.AP) else handle_or_ap
    return bass.AP(tensor=a0.tensor, offset=offset, ap=[list(d) for d in dims])


def _split_multi_waits(nc):
    """This toolchain's walrus accepts at most ONE embedded on_wait per
    instruction; hoist extras into standalone EventSemaphore waits just
    before the instruction on the same engine."""
    n = 0
    for bb in nc.main_func.blocks:
        new = []
        for ins in bb.instructions:
            si = ins.sync_info
            if si is not None and si.on_wait and len(si.on_wait) > 1:
                for w in si.on_wait[:-1]:
                    n += 1
                    wi = mybir.InstEventSemaphore(
                        name=f"I-waitsplit-{n}",
                        opcode="EventSemaphore",
                        engine=ins.engine,
                        sync_info=mybir.SyncInfo(on_wait=[w], on_update=[]),
                    )
                    try:
                        nc.register_instruction(wi)
                    except Exception:
                        pass
                    new.append(wi)
                si.on_wait = si.on_wait[-1:]
            new.append(ins)
        try:
            bb.instructions[:] = new
        except TypeError:
            bb.instructions = new
    return n


def build_nc():
    nc = bass.Bass()

    xT = nc.dram_tensor("xT", [E, S], F32, kind="ExternalInput")
    wT = nc.dram_tensor("wT", [E, P2], F32, kind="ExternalInput")
    bvec = nc.dram_tensor("bvec", [P2, 1], F32, kind="ExternalInput")
    prior = nc.dram_tensor("prior", [S, S], F32, kind="ExternalInput")
    na_out = nc.dram_tensor("na_out", [S, S], F32, kind="ExternalOutput")
    c_out = nc.dram_tensor("c_out", [S, S], F32, kind="ExternalOutput")

    # Constants baked into the NEFF. Window masks are [128,130]: for row-block
    # r the band lives in absolute cols [r*128-1, r*128+129); with window
    # origin w0 = r*128-1 the super-diag sits at rel col p+2, diag at p+1,
    # sub-diag at p, independent of r.
    p_i = np.arange(128)[:, None]
    c_i = np.arange(130)[None, :]
    mu_h = nc.inline_tensor((c_i == p_i + 2).astype(np.float32), "mask_u")
    ml_h = nc.inline_tensor((c_i == p_i).astype(np.float32), "mask_l")
    md_h = nc.inline_tensor((c_i == p_i + 1).astype(np.float32), "mask_d")
    m1d_h = nc.inline_tensor((c_i != p_i + 1).astype(np.float32), "mask_1md")
    # lhsT for within-block inclusive cumsum over partitions: out = triu.T @ u
    triu_h = nc.inline_tensor(
        np.triu(np.ones((128, 128), np.float32)), "triu_ones"
    )
    neg_h = nc.inline_tensor(np.full((1,), -1e30, np.float32), "neg_inf")
    zer_h = nc.inline_tensor(np.zeros((1,), np.float32), "zero_one")

    with tile.TileContext(nc) as tc:
        with (
            tc.tile_pool(name="setup", bufs=1) as setup,
            tc.tile_pool(name="blk", bufs=3) as blk,
            tc.tile_pool(name="mm", bufs=2, space="PSUM") as mm,
            tc.tile_pool(name="mm1", bufs=1, space="PSUM") as mm1,
            tc.tile_pool(name="ps_small", bufs=1, space="PSUM") as ps_small,
            tc.tile_pool(name="dram", bufs=1, space="DRAM") as dram,
        ):
            # ---------------- constants / weights into SBUF ----------------
            mu_t = setup.tile([128, 130], F32)
            nc.sync.dma_start(out=mu_t, in_=mu_h[:])
            ml_t = setup.tile([128, 130], F32)
            nc.sync.dma_start(out=ml_t, in_=ml_h[:])
            md_t = setup.tile([128, 130], F32)
            nc.sync.dma_start(out=md_t, in_=md_h[:])
            m1d_t = setup.tile([128, 130], F32)
            nc.sync.dma_start(out=m1d_t, in_=m1d_h[:])
            triu_t = setup.tile([128, 128], F32)
            nc.sync.dma_start(out=triu_t, in_=triu_h[:])

            wT_t = setup.tile([128, 4, P2], F32)
            nc.sync.dma_start(
                out=wT_t,
                in_=_ap(wT, 0, [[P2, 128], [128 * P2, 4], [1, P2]]))
            biasq_t = setup.tile([64, 1], F32)
            nc.sync.dma_start(out=biasq_t, in_=bvec[0:64, :])
            biask_t = setup.tile([64, 1], F32)
            nc.sync.dma_start(out=biask_t, in_=bvec[64:128, :])

            xT_t = setup.tile([128, 4, S], F32)
            nc.sync.dma_start(
                out=xT_t,
                in_=_ap(xT, 0, [[S, 128], [128 * S, 4], [1, S]]))

            # ---------------- qT/kT = (x @ W.T).T halves  [64, S] ----------
            qT_t = setup.tile([64, S], F32)
            kT_t = setup.tile([64, S], F32)
            for j in range(2):
                for half, (dest_t, bias_t) in enumerate(
                        ((qT_t, biasq_t), (kT_t, biask_t))):
                    ps = mm.tile([64, 512], F32, tag="mmbig")
                    for c in range(4):
                        nc.tensor.matmul(
                            ps,
                            lhsT=wT_t[:, c, half * 64:(half + 1) * 64],
                            rhs=xT_t[:, c, j * 512:(j + 1) * 512],
                            start=(c == 0),
                            stop=(c == 3),
                        )
                    nc.vector.tensor_scalar_add(
                        dest_t[:, j * 512:(j + 1) * 512], ps, bias_t)

            # adjacent-pair products; reduce over P=64 partitions via ones-matmul
            tu_t = setup.tile([64, S - 1], F32)
            nc.vector.tensor_mul(tu_t, qT_t[:, 0:S - 1], kT_t[:, 1:S])
            tl_t = setup.tile([64, S - 1], F32)
            nc.vector.tensor_mul(tl_t, qT_t[:, 1:S], kT_t[:, 0:S - 1])
            ones_t = setup.tile([64, 1], F32)
            nc.vector.memset(ones_t, 1.0)

            su_ps = mm1.tile([1, S], F32, tag="su")
            sl_ps = mm1.tile([1, S], F32, tag="sl")
            # s_u[i] = q_i.k_{i+1} at col i (i<=1022); col 1023 garbage
            nc.tensor.matmul(su_ps[0:1, 0:512], lhsT=ones_t, rhs=tu_t[:, 0:512],
                             start=True, stop=True)
            nc.tensor.matmul(su_ps[0:1, 512:1023], lhsT=ones_t, rhs=tu_t[:, 512:1023],
                             start=True, stop=True)
            # s_l[i] = q_i.k_{i-1} at col i (i>=1); col 0 garbage
            nc.tensor.matmul(sl_ps[0:1, 1:512], lhsT=ones_t, rhs=tl_t[:, 0:511],
                             start=True, stop=True)
            nc.tensor.matmul(sl_ps[0:1, 512:1024], lhsT=ones_t, rhs=tl_t[:, 511:1023],
                             start=True, stop=True)

            # reshape linear [1,1024] -> [128 part, 8 blk]  (i = r*128 + p)
            su_sb = setup.tile([1, S], F32)
            nc.vector.tensor_copy(su_sb[:, 0:S - 1], su_ps[0:1, 0:S - 1])
            nc.vector.memset(su_sb[:, S - 1:S], -1e30)   # row 1023: no super-diag
            sl_sb = setup.tile([1, S], F32)
            nc.vector.tensor_copy(sl_sb[:, 1:S], sl_ps[0:1, 1:S])
            nc.vector.memset(sl_sb[:, 0:1], -1e30)       # row 0: no sub-diag
            su_lin = dram.tile([S], F32)
            nc.sync.dma_start(out=su_lin[:], in_=su_sb)
            sl_lin = dram.tile([S], F32)
            nc.sync.dma_start(out=sl_lin[:], in_=sl_sb)
            s_u = setup.tile([128, NB], F32)
            nc.sync.dma_start(out=s_u, in_=_ap(su_lin[:], 0, [[1, 128], [128, NB]]))
            s_l = setup.tile([128, NB], F32)
            nc.sync.dma_start(out=s_l, in_=_ap(sl_lin[:], 0, [[1, 128], [128, NB]]))

            # ---------------- 2-element softmax ----------------
            m_t = setup.tile([128, NB], F32)
            nc.vector.tensor_max(m_t, s_u, s_l)
            du_t = setup.tile([128, NB], F32)
            nc.vector.tensor_sub(du_t, s_u, m_t)
            dl_t = setup.tile([128, NB], F32)
            nc.vector.tensor_sub(dl_t, s_l, m_t)
            eu_t = setup.tile([128, NB], F32)
            nc.scalar.activation(eu_t, du_t, AF.Exp, scale=1.0 / E)
            el_t = setup.tile([128, NB], F32)
            nc.scalar.activation(el_t, dl_t, AF.Exp, scale=1.0 / E)
            ssum_t = setup.tile([128, NB], F32)
            nc.vector.tensor_add(ssum_t, eu_t, el_t)
            rs_t = setup.tile([128, NB], F32)
            nc.vector.reciprocal(rs_t, ssum_t)
            a_u = setup.tile([128, NB], F32)
            nc.vector.tensor_mul(a_u, eu_t, rs_t)
            a_l = setup.tile([128, NB], F32)
            nc.vector.tensor_mul(a_l, el_t, rs_t)

            # a_l shifted to i+1 (partition shifts via DMA)
            a_ln = setup.tile([128, NB], F32)
            nc.sync.dma_start(out=a_ln[0:127, :], in_=a_l[1:128, :])
            nc.sync.dma_start(out=a_ln[127:128, 0:7], in_=a_l[0:1, 1:8])
            nc.sync.dma_start(out=a_ln[127:128, 7:8], in_=zer_h[:])  # i=1023 unused

            eps_t = setup.tile([128, 1], F32)
            nc.vector.memset(eps_t, 1e-9)
            gq_t = setup.tile([128, NB], F32)
            nc.vector.tensor_mul(gq_t, a_u, a_ln)
            g_u = setup.tile([128, NB], F32)
            nc.scalar.activation(g_u, gq_t, AF.Sqrt, bias=eps_t[:, 0:1])
            g_l = setup.tile([128, NB], F32)          # g_l[i] = g_u[i-1]
            nc.sync.dma_start(out=g_l[1:128, :], in_=g_u[0:127, :])
            nc.sync.dma_start(out=g_l[0:1, 1:8], in_=g_u[127:128, 0:7])
            nc.vector.memset(g_l[0:1, 0:1], C0)          # row 0 unused

            # ---------------- prior band gathers (all writes disjoint) ------
            pr_u = setup.tile([128, NB], F32)            # prior[i, i+1]
            nc.gpsimd.dma_start(
                out=pr_u[:, 0:7],
                in_=_ap(prior, 1, [[S + 1, 128], [128 * (S + 1), 7]]))
            nc.gpsimd.dma_start(
                out=pr_u[0:127, 7:8],
                in_=_ap(prior, 896 * (S + 1) + 1, [[S + 1, 127], [1, 1]]))
            nc.gpsimd.dma_start(out=pr_u[127:128, 7:8], in_=zer_h[:])
            pr_l = setup.tile([128, NB], F32)            # prior[i, i-1]
            nc.gpsimd.dma_start(
                out=pr_l[1:128, 0:1],
                in_=_ap(prior, S, [[S + 1, 127], [1, 1]]))
            nc.gpsimd.dma_start(
                out=pr_l[:, 1:8],
                in_=_ap(prior, 128 * (S + 1) - 1, [[S + 1, 128], [128 * (S + 1), 7]]))
            nc.gpsimd.dma_start(out=pr_l[0:1, 0:1], in_=zer_h[:])
            pr_d = setup.tile([128, NB], F32)            # prior[i, i]
            nc.gpsimd.dma_start(
                out=pr_d, in_=_ap(prior, 0, [[S + 1, 128], [128 * (S + 1), 8]]))

            # band values of neighbor_attn
            omg_t = setup.tile([128, NB], F32)           # 1 - g_u
            nc.vector.tensor_scalar(omg_t, g_u, -1.0, 1.0, op0=ALU.mult, op1=ALU.add)
            na_bu = setup.tile([128, NB], F32)           # na[i, i+1]
            t_tmp = setup.tile([128, NB], F32)
            nc.vector.tensor_mul(t_tmp, pr_u, omg_t)
            nc.vector.tensor_add(na_bu, t_tmp, g_u)
            omgl_t = setup.tile([128, NB], F32)          # 1 - g_l
            nc.vector.tensor_scalar(omgl_t, g_l, -1.0, 1.0, op0=ALU.mult, op1=ALU.add)
            na_bl = setup.tile([128, NB], F32)           # na[i, i-1]
            t2_tmp = setup.tile([128, NB], F32)
            nc.vector.tensor_mul(t2_tmp, pr_l, omgl_t)
            nc.vector.tensor_add(na_bl, t2_tmp, g_l)
            nd_t = setup.tile([128, NB], F32)            # na[i, i]
            nc.scalar.activation(nd_t, pr_d, AF.Copy, bias=C0, scale=1.0 - C0)

            # ---------------- U = exclusive prefix sum of u ----------------
            u_t = setup.tile([128, NB], F32)
            nc.scalar.activation(u_t, na_bu, AF.Ln, bias=eps_t[:, 0:1])
            inc_ps = ps_small.tile([128, NB], F32)
            nc.tensor.matmul(inc_ps, lhsT=triu_t, rhs=u_t, start=True, stop=True)
            exc_t = setup.tile([128, NB], F32)
            nc.vector.tensor_sub(exc_t, inc_ps, u_t)

            bs_sb = setup.tile([128, NB], F32)
            nc.vector.tensor_copy(bs_sb, inc_ps)
            bs_t = setup.tile([1, NB], F32)              # per-block sums -> part 0
            nc.sync.dma_start(out=bs_t, in_=bs_sb[127:128, :])
            bp_t = setup.tile([1, NB], F32)
            nc.vector.memset(bp_t[:, 0:1], 0.0)
            nc.vector.tensor_copy(bp_t[:, 1:8], bs_t[:, 0:7])
            zer_t = setup.tile([1, NB], F32)
            nc.vector.memset(zer_t, 0.0)
            bpx_t = setup.tile([1, NB], F32)             # exclusive block prefix
            nc.vector.tensor_tensor_scan(bpx_t, bp_t, zer_t, 0.0,
                                         op0=ALU.add, op1=ALU.add)

            bp_d = dram.tile([NB], F32)
            nc.sync.dma_start(out=bp_d[:], in_=bpx_t)
            bp_rep = setup.tile([128, NB], F32)
            nc.sync.dma_start(out=bp_rep, in_=_ap(bp_d[:], 0, [[0, 128], [1, NB]]))
            U_t = setup.tile([128, NB], F32)
            nc.vector.tensor_add(U_t, exc_t, bp_rep)

            u_lin = dram.tile([S], F32)
            nc.sync.dma_start(out=_ap(u_lin[:], 0, [[1, 128], [128, NB]]), in_=U_t)
            U_rep = setup.tile([128, S], F32)
            nc.sync.dma_start(out=U_rep, in_=_ap(u_lin[:], 0, [[0, 128], [1, S]]))

            gu_c = setup.tile([128, NB], F32)
            nc.vector.tensor_scalar_sub(gu_c, g_u, C0)
            gl_c = setup.tile([128, NB], F32)
            nc.vector.tensor_scalar_sub(gl_c, g_l, C0)

            # ---------------- main per-row-block loop ----------------
            for r in range(NB):
                w0 = r * 128 - 1
                wlo = max(w0, 0)
                whi = min(w0 + 130, S)
                wd = whi - wlo
                mo = wlo - w0

                pr_t = blk.tile([128, S], F32, tag="pr")
                nc.sync.dma_start(out=pr_t, in_=prior[r * 128:(r + 1) * 128, :])

                na_t = blk.tile([128, S], F32, tag="na")
                nc.scalar.activation(na_t, pr_t, AF.Copy, bias=C0, scale=1.0 - C0)

                # band window: g = C0 + M_u*(g_u-C0) + M_l*(g_l-C0)
                gwin = blk.tile([128, 130], F32, tag="gwin")
                nc.vector.tensor_scalar(gwin[:, :wd], mu_t[:, mo:mo + wd],
                                        gu_c[:, r:r + 1], C0,
                                        op0=ALU.mult, op1=ALU.add)
                t2w = blk.tile([128, 130], F32, tag="t2w")
                nc.vector.tensor_scalar(t2w[:, :wd], ml_t[:, mo:mo + wd],
                                        gl_c[:, r:r + 1], None, op0=ALU.mult)
                gw2 = blk.tile([128, 130], F32, tag="gw2")
                nc.vector.tensor_add(gw2[:, :wd], gwin[:, :wd], t2w[:, :wd])
                # na_win = g + prior*(1-g) = g + prior - prior*g
                t3w = blk.tile([128, 130], F32, tag="t3w")
                nc.vector.tensor_mul(t3w[:, :wd], pr_t[:, wlo:whi], gw2[:, :wd])
                t4w = blk.tile([128, 130], F32, tag="t4w")
                nc.vector.tensor_sub(t4w[:, :wd], pr_t[:, wlo:whi], t3w[:, :wd])
                nc.vector.tensor_add(na_t[:, wlo:whi], t4w[:, :wd], gw2[:, :wd])

                nc.sync.dma_start(out=na_out[r * 128:(r + 1) * 128, :], in_=na_t)

                # c_attn block: exp(-|U[j] - U[i]|), diag <- na[i,i]
                c_t = blk.tile([128, S], F32, tag="ct")
                nc.vector.tensor_scalar(c_t, U_rep, U_t[:, r:r + 1], None,
                                        op0=ALU.subtract)
                cn_t = blk.tile([128, S], F32, tag="cn")
                nc.vector.tensor_scalar(cn_t, c_t, -1.0, None, op0=ALU.mult)
                cm_t = blk.tile([128, S], F32, tag="cm")
                nc.vector.tensor_max(cm_t, c_t, cn_t)
                c2_t = blk.tile([128, S], F32, tag="c2")
                nc.scalar.activation(c2_t, cm_t, AF.Exp, scale=-1.0)
                t5w = blk.tile([128, 130], F32, tag="t5w")
                nc.vector.tensor_scalar(t5w[:, :wd], md_t[:, mo:mo + wd],
                                        nd_t[:, r:r + 1], None, op0=ALU.mult)
                t6w = blk.tile([128, 130], F32, tag="t6w")
                nc.vector.tensor_mul(t6w[:, :wd], c2_t[:, wlo:whi],
                                     m1d_t[:, mo:mo + wd])
                nc.vector.tensor_add(c2_t[:, wlo:whi], t5w[:, :wd], t6w[:, :wd])

                nc.sync.dma_start(out=c_out[r * 128:(r + 1) * 128, :], in_=c2_t)

    _split_multi_waits(nc)
    return nc


def _get_nc():
    if "nc" not in _CACHE:
        _CACHE["nc"] = build_nc()
    return _CACHE["nc"]


def run(inputs, trace=False):
    nc = _get_nc()
    context = np.asarray(inputs["context"], np.float32)
    prior = np.asarray(inputs["prior"], np.float32)
    w = np.asarray(inputs["proj_weight"], np.float32)
    bias = np.asarray(inputs["proj_bias"], np.float32)

    wT = np.ascontiguousarray(w.T)                     # [E, 2P]
    bcol = np.ascontiguousarray(bias.reshape(P2, 1))
    in_maps = []
    for b in range(B):
        in_maps.append({
            "xT": np.ascontiguousarray(context[:, b, :].T),   # [E, S]
            "wT": wT,
            "bvec": bcol,
            "prior": np.ascontiguousarray(prior[b]),
        })
    try:
        res = run_bass_kernel_spmd(nc, in_maps, list(range(B)), trace=trace)
    except ModuleNotFoundError:
        res = run_bass_kernel_spmd(nc, in_maps, list(range(B)), trace=False)
    c = np.stack([res.results[i]["c_out"] for i in range(B)])
    na = np.stack([res.results[i]["na_out"] for i in range(B)])
    return (c, na), res


def kernel(**inputs):
    (c, na), _ = run(inputs)
    return (c, na)



# revision 6
# speedup vs baseline: 1.4978x; 1.4978x over previous
"""ConstituentAttention Trainium2 kernel.

Math (derived from the reference):
  - score is masked to the super/sub-diagonal only, so the row softmax is a
    2-element softmax: a_u[i] = sigmoid((s_u[i]-s_l[i])/E), a_l = 1-a_u,
    where s_u[i] = q_i.k_{i+1}, s_l[i] = q_i.k_{i-1}.
  - neighbor_attn = prior + (1-prior)*g where g == sqrt(1e-9) =: C0 everywhere
    except g[i,i+1] = g[i+1,i] = sqrt(a_u[i]*a_l[i+1] + 1e-9) =: g_u[i].
  - log-space prefix products collapse to c_attn[i,j] = exp(-|U[j]-U[i]|) for
    i != j, where U = exclusive prefix sum of u_i = log(na[i,i+1] + 1e-9);
    diagonal of c_attn = na[i,i].

Sharding: data-parallel over batch, one batch element per NeuronCore (B=8).

Layout notes: i = r*128 + p (partition p fast, block r = 0..7 slow), so the
per-index arrays live as [128, 8] SBUF tiles.  s_u/s_l are extracted from
[1, S]-ish linear staging rows with THREE free-dim offsets (i-1, i, i+1)
stacked as [128, 24] tiles, which turns every partition-shift the algorithm
needs into a free-dim offset.
"""

import numpy as np

import concourse.bass as bass
import concourse.tile as tile
from concourse import mybir
from concourse.bass_utils import run_bass_kernel_spmd

S, B, E, P = 1024, 8, 512, 64
P2 = 2 * P
NB = S // 128
C0 = float(np.sqrt(1e-9))
NEG = -1e30
F32 = mybir.dt.float32
F32R = mybir.dt.float32r
AF = mybir.ActivationFunctionType
ALU = mybir.AluOpType

_CACHE = {}


def _ap(handle_or_ap, offset, dims):
    a0 = handle_or_ap[:] if not isinstance(handle_or_ap, bass.AP) else handle_or_ap
    return bass.AP(tensor=a0.tensor, offset=offset, ap=[list(d) for d in dims])


def _r(ap):
    return ap.bitcast(F32R)


def _split_multi_waits(nc):
    """This toolchain's walrus accepts at most ONE embedded on_wait per
    instruction; hoist extras into standalone EventSemaphore waits just
    before the instruction on the same engine."""
    n = 0
    for bb in nc.main_func.blocks:
        new = []
        for ins in bb.instructions:
            si = ins.sync_info
            if si is not None and si.on_wait and len(si.on_wait) > 1:
                for w in si.on_wait[:-1]:
                    n += 1
                    wi = mybir.InstEventSemaphore(
                        name=f"I-waitsplit-{n}",
                        opcode="EventSemaphore",
                        engine=ins.engine,
                        sync_info=mybir.SyncInfo(on_wait=[w], on_update=[]),
                    )
                    try:
                        nc.register_instruction(wi)
                    except Exception:
                        pass
                    new.append(wi)
                si.on_wait = si.on_wait[-1:]
            new.append(ins)
        try:
            bb.instructions[:] = new
        except TypeError:
            bb.instructions = new
    return n


def build_nc():
    nc = bass.Bass()

    xT = nc.dram_tensor("xT", [E, S], F32, kind="ExternalInput")
    wT = nc.dram_tensor("wT", [E, P2], F32, kind="ExternalInput")
    bvec = nc.dram_tensor("bvec", [P2, 1], F32, kind="ExternalInput")
    prior = nc.dram_tensor("prior", [S, S], F32, kind="ExternalInput")
    na_out = nc.dram_tensor("na_out", [S, S], F32, kind="ExternalOutput")
    c_out = nc.dram_tensor("c_out", [S, S], F32, kind="ExternalOutput")

    # Window masks are [128,130]: for row-block r the band lives in absolute
    # cols [r*128-1, r*128+129); with window origin w0 = r*128-1 the super-diag
    # sits at rel col p+2, diag at p+1, sub-diag at p, independent of r.
    p_i = np.arange(128)[:, None]
    c_i = np.arange(130)[None, :]
    mu_h = nc.inline_tensor((c_i == p_i + 2).astype(np.float32), "mask_u")
    ml_h = nc.inline_tensor((c_i == p_i).astype(np.float32), "mask_l")
    md_h = nc.inline_tensor((c_i == p_i + 1).astype(np.float32), "mask_d")
    m1d_h = nc.inline_tensor((c_i != p_i + 1).astype(np.float32), "mask_1md")
    # lhsT for within-block inclusive cumsum over partitions: out = triu.T @ u
    triu_h = nc.inline_tensor(
        np.triu(np.ones((128, 128), np.float32)), "triu_ones"
    )
    ones_col_h = nc.inline_tensor(np.ones((128, 1), np.float32), "ones_col")
    ones_row_h = nc.inline_tensor(np.ones((1, 128), np.float32), "ones_row")

    with tile.TileContext(nc) as tc:
        with (
            tc.tile_pool(name="setup", bufs=1) as setup,
            tc.tile_pool(name="blk", bufs=3) as blk,
            tc.tile_pool(name="mm", bufs=2, space="PSUM") as mm,
            tc.tile_pool(name="mm1", bufs=2, space="PSUM") as mm1,
            tc.tile_pool(name="ps_small", bufs=2, space="PSUM") as ps_small,
            tc.tile_pool(name="psrep", bufs=1, space="PSUM") as psrep,
        ):
            # ---------------- constants / weights into SBUF ----------------
            mu_t = setup.tile([128, 130], F32)
            nc.sync.dma_start(out=mu_t, in_=mu_h[:])
            ml_t = setup.tile([128, 130], F32)
            nc.sync.dma_start(out=ml_t, in_=ml_h[:])
            md_t = setup.tile([128, 130], F32)
            nc.sync.dma_start(out=md_t, in_=md_h[:])
            m1d_t = setup.tile([128, 130], F32)
            nc.sync.dma_start(out=m1d_t, in_=m1d_h[:])
            triu_t = setup.tile([128, 128], F32)
            nc.sync.dma_start(out=triu_t, in_=triu_h[:])
            ones_col = setup.tile([128, 1], F32)
            nc.sync.dma_start(out=ones_col, in_=ones_col_h[:])
            ones_row = setup.tile([1, 128], F32)
            nc.sync.dma_start(out=ones_row, in_=ones_row_h[:])

            wT_t = setup.tile([128, 4, P2], F32)
            nc.sync.dma_start(
                out=wT_t,
                in_=_ap(wT, 0, [[P2, 128], [128 * P2, 4], [1, P2]]))
            biasq_t = setup.tile([64, 1], F32)
            nc.sync.dma_start(out=biasq_t, in_=bvec[0:64, :])
            biask_t = setup.tile([64, 1], F32)
            nc.sync.dma_start(out=biask_t, in_=bvec[64:128, :])

            xT_t = setup.tile([128, 4, S], F32)
            nc.sync.dma_start(
                out=xT_t,
                in_=_ap(xT, 0, [[S, 128], [128 * S, 4], [1, S]]))

            # ------- prior band gathers (early; feed U chain + c diag) ------
            pr_u = setup.tile([128, NB], F32)            # prior[i, i+1]
            nc.gpsimd.dma_start(
                out=pr_u[:, 0:7],
                in_=_ap(prior, 1, [[S + 1, 128], [128 * (S + 1), 7]]))
            nc.gpsimd.dma_start(
                out=pr_u[0:127, 7:8],
                in_=_ap(prior, 896 * (S + 1) + 1, [[S + 1, 127], [1, 1]]))
            nc.vector.memset(pr_u[127:128, 7:8], 0.0)    # i=1023: no (i,i+1)
            pr_d = setup.tile([128, NB], F32)            # prior[i, i]
            nc.gpsimd.dma_start(
                out=pr_d, in_=_ap(prior, 0, [[S + 1, 128], [128 * (S + 1), 8]]))

            # ---------------- qT/kT = (x @ W.T).T halves  [64, S] ----------
            # fp32r matmuls: out free 512 >= 256 -> 1 cycle/row.
            qT_t = setup.tile([64, S], F32)
            kT_t = setup.tile([64, S], F32)
            for j in range(2):
                for half, (dest_t, bias_t) in enumerate(
                        ((qT_t, biasq_t), (kT_t, biask_t))):
                    ps = mm.tile([64, 512], F32, tag="mmbig")
                    for c in range(4):
                        nc.tensor.matmul(
                            ps[:],
                            lhsT=_r(wT_t[:, c, half * 64:(half + 1) * 64]),
                            rhs=_r(xT_t[:, c, j * 512:(j + 1) * 512]),
                            start=(c == 0),
                            stop=(c == 3),
                        )
                    nc.vector.tensor_scalar_add(
                        dest_t[:, j * 512:(j + 1) * 512], ps, bias_t)

            # adjacent-pair products; reduce over P=64 partitions via ones-mm
            tu_t = setup.tile([64, S - 1], F32)
            nc.vector.tensor_mul(tu_t, qT_t[:, 0:S - 1], kT_t[:, 1:S])
            tl_t = setup.tile([64, S - 1], F32)
            nc.vector.tensor_mul(tl_t, qT_t[:, 1:S], kT_t[:, 0:S - 1])

            # linear staging rows with -1e30 pads; index maps:
            #   su_stage[k] = s_u[k-1]   (valid s_u: 0..1022; s_u[1023]=-inf)
            #   sl_stage[k] = s_l[k-1] = tl[k-2]  (s_l[0]=-inf)
            # su[i] = s_u[i] = q_i.k_{i+1} = tu[i]; tl[j] = s_l[j+1].
            su_st = setup.tile([1, 1026], F32)
            nc.vector.memset(su_st[:, 0:1], NEG)
            nc.vector.memset(su_st[:, 1024:1026], NEG)
            sl_st = setup.tile([1, 1026], F32)
            nc.vector.memset(sl_st[:, 0:2], NEG)
            nc.vector.memset(sl_st[:, 1025:1026], NEG)
            for src_t, st_t, off in ((tu_t, su_st, 1), (tl_t, sl_st, 2)):
                for lo in (0, 512):
                    w = min(lo + 512, S - 1) - lo
                    ps1 = mm1.tile([1, 512], F32, tag="ones")
                    nc.tensor.matmul(ps1[0:1, 0:w], lhsT=_r(ones_col[0:64, :]),
                                     rhs=_r(src_t[:, lo:lo + w]),
                                     start=True, stop=True)
                    nc.vector.tensor_copy(st_t[:, off + lo:off + lo + w],
                                          ps1[0:1, 0:w])

            # [128, 24] stacks: col groups g=0,1,2 hold index offsets i-1,i,i+1
            s_uu = setup.tile([128, 3, NB], F32)
            s_ll = setup.tile([128, 3, NB], F32)
            for g in range(3):
                nc.sync.dma_start(
                    out=s_uu[:, g, :],
                    in_=_ap(su_st[:], g, [[1, 128], [128, NB]]))
                nc.sync.dma_start(
                    out=s_ll[:, g, :],
                    in_=_ap(sl_st[:], g, [[1, 128], [128, NB]]))

            # 2-element softmax via sigmoid on all 3 offset groups at once
            diff_t = setup.tile([128, 3, NB], F32)
            nc.vector.tensor_sub(diff_t, s_uu, s_ll)
            a_u = setup.tile([128, 3, NB], F32)
            nc.scalar.activation(a_u, diff_t, AF.Sigmoid, scale=1.0 / E)
            a_l = setup.tile([128, 3, NB], F32)
            nc.scalar.activation(a_l, diff_t, AF.Sigmoid, scale=-1.0 / E)

            # g_l[i] = g_u[i-1] = sqrt(a_u[i-1]*a_l[i] + eps)  (cols 0:8)
            # g_u[i]            = sqrt(a_u[i]*a_l[i+1] + eps)  (cols 8:16)
            gq_t = setup.tile([128, 2, NB], F32)
            nc.vector.tensor_mul(gq_t, _ap(a_u[:], 0, [[24, 128], [8, 2], [1, NB]]),
                                 _ap(a_l[:], 8, [[24, 128], [8, 2], [1, NB]]))
            eps_t = setup.tile([128, 1], F32)
            nc.vector.memset(eps_t, 1e-9)
            g_t = setup.tile([128, 2, NB], F32)
            nc.scalar.activation(g_t, gq_t, AF.Sqrt, bias=eps_t[:, 0:1])
            g_l = g_t[:, 0, :]
            g_u = g_t[:, 1, :]

            # gu_c/gl_c = g - C0 for the banded block-loop update
            gc_t = setup.tile([128, 2, NB], F32)
            nc.vector.tensor_scalar_sub(gc_t, g_t, C0)
            gl_c = gc_t[:, 0, :]
            gu_c = gc_t[:, 1, :]

            # na[i,i+1] = g_u + pr_u*(1-g_u);  u = ln(na + eps)
            omg_t = setup.tile([128, NB], F32)
            nc.vector.tensor_scalar(omg_t, g_u, -1.0, 1.0, op0=ALU.mult,
                                    op1=ALU.add)
            t_tmp = setup.tile([128, NB], F32)
            nc.vector.tensor_mul(t_tmp, pr_u, omg_t)
            na_bu = setup.tile([128, NB], F32)
            nc.vector.tensor_add(na_bu, t_tmp, g_u)
            u_t = setup.tile([128, NB], F32)
            nc.scalar.activation(u_t, na_bu, AF.Ln, bias=eps_t[:, 0:1])
            nd_t = setup.tile([128, NB], F32)            # na[i, i]
            nc.scalar.activation(nd_t, pr_d, AF.Copy, bias=C0, scale=1.0 - C0)

            # ---- U = exclusive prefix sum of u (no DRAM round trips) ----
            inc_ps = ps_small.tile([128, NB], F32, tag="tiny")
            nc.tensor.matmul(inc_ps, lhsT=triu_t, rhs=u_t, start=True, stop=True)
            exc_t = setup.tile([128, NB], F32)
            nc.vector.tensor_sub(exc_t, inc_ps, u_t)

            cs_ps = ps_small.tile([1, NB], F32, tag="tiny")   # per-block sums
            nc.tensor.matmul(cs_ps, lhsT=ones_col, rhs=u_t, start=True, stop=True)
            bp_t = setup.tile([1, NB], F32)
            nc.vector.memset(bp_t[:, 0:1], 0.0)
            nc.vector.tensor_copy(bp_t[:, 1:8], cs_ps[0:1, 0:7])
            zer_t = setup.tile([1, NB], F32)
            nc.vector.memset(zer_t, 0.0)
            bpx_t = setup.tile([1, NB], F32)             # exclusive block prefix
            nc.vector.tensor_tensor_scan(bpx_t, bp_t, zer_t, 0.0,
                                         op0=ALU.add, op1=ALU.add)
            bpr_ps = ps_small.tile([128, NB], F32, tag="tiny")
            nc.tensor.matmul(bpr_ps, lhsT=ones_row, rhs=bpx_t, start=True,
                             stop=True)
            U_t = setup.tile([128, NB], F32)
            nc.vector.tensor_add(U_t, exc_t, bpr_ps)

            # U_rep[p, j] = U[j] via SBUF reshape DMA + ones broadcast matmul
            U_lin = setup.tile([1, S], F32)
            nc.sync.dma_start(out=_ap(U_lin[:], 0, [[1, 128], [128, NB]]),
                              in_=U_t)
            Ur_ps = psrep.tile([128, S], F32, tag="urep")
            for lo in (0, 512):
                nc.tensor.matmul(Ur_ps[:, lo:lo + 512], lhsT=_r(ones_row),
                                 rhs=_r(U_lin[0:1, lo:lo + 512]), start=True,
                                 stop=True)

            # ---------------- main per-row-block loop ----------------
            for r in range(NB):
                w0 = r * 128 - 1
                wlo = max(w0, 0)
                whi = min(w0 + 130, S)
                wd = whi - wlo
                mo = wlo - w0

                pr_t = blk.tile([128, S], F32, tag="pr")
                nc.sync.dma_start(out=pr_t, in_=prior[r * 128:(r + 1) * 128, :])

                na_t = blk.tile([128, S], F32, tag="na")
                nc.scalar.activation(na_t, pr_t, AF.Copy, bias=C0, scale=1.0 - C0)

                # band window: g = C0 + M_u*(g_u-C0) + M_l*(g_l-C0)
                gwin = blk.tile([128, 130], F32, tag="gwin")
                nc.vector.tensor_scalar(gwin[:, :wd], mu_t[:, mo:mo + wd],
                                        gu_c[:, r:r + 1], C0,
                                        op0=ALU.mult, op1=ALU.add)
                t2w = blk.tile([128, 130], F32, tag="t2w")
                nc.vector.tensor_scalar(t2w[:, :wd], ml_t[:, mo:mo + wd],
                                        gl_c[:, r:r + 1], None, op0=ALU.mult)
                gw2 = blk.tile([128, 130], F32, tag="gw2")
                nc.vector.tensor_add(gw2[:, :wd], gwin[:, :wd], t2w[:, :wd])
                # na_win = g + prior*(1-g) = g + prior - prior*g
                t3w = blk.tile([128, 130], F32, tag="t3w")
                nc.vector.tensor_mul(t3w[:, :wd], pr_t[:, wlo:whi], gw2[:, :wd])
                t4w = blk.tile([128, 130], F32, tag="t4w")
                nc.vector.tensor_sub(t4w[:, :wd], pr_t[:, wlo:whi], t3w[:, :wd])
                nc.vector.tensor_add(na_t[:, wlo:whi], t4w[:, :wd], gw2[:, :wd])

                nc.sync.dma_start(out=na_out[r * 128:(r + 1) * 128, :], in_=na_t)

                # c_attn block: exp(-|U[j] - U[i]|), diag <- na[i,i]
                cd_t = blk.tile([128, S], F32, tag="cd")
                nc.vector.tensor_scalar(cd_t, Ur_ps, U_t[:, r:r + 1], 0.0,
                                        op0=ALU.subtract, op1=ALU.abs_max)
                c2_t = blk.tile([128, S], F32, tag="c2")
                nc.scalar.activation(c2_t, cd_t, AF.Exp, scale=-1.0)
                t5w = blk.tile([128, 130], F32, tag="t5w")
                nc.vector.tensor_scalar(t5w[:, :wd], md_t[:, mo:mo + wd],
                                        nd_t[:, r:r + 1], None, op0=ALU.mult)
                t6w = blk.tile([128, 130], F32, tag="t6w")
                nc.vector.tensor_mul(t6w[:, :wd], c2_t[:, wlo:whi],
                                     m1d_t[:, mo:mo + wd])
                nc.vector.tensor_add(c2_t[:, wlo:whi], t5w[:, :wd], t6w[:, :wd])

                nc.sync.dma_start(out=c_out[r * 128:(r + 1) * 128, :], in_=c2_t)

    _split_multi_waits(nc)
    return nc


def _get_nc():
    if "nc" not in _CACHE:
        _CACHE["nc"] = build_nc()
    return _CACHE["nc"]


def run(inputs, trace=False):
    nc = _get_nc()
    context = np.asarray(inputs["context"], np.float32)
    prior = np.asarray(inputs["prior"], np.float32)
    w = np.asarray(inputs["proj_weight"], np.float32)
    bias = np.asarray(inputs["proj_bias"], np.float32)

    wT = np.ascontiguousarray(w.T)                     # [E, 2P]
    bcol = np.ascontiguousarray(bias.reshape(P2, 1))
    in_maps = []
    for b in range(B):
        in_maps.append({
            "xT": np.ascontiguousarray(context[:, b, :].T),   # [E, S]
            "wT": wT,
            "bvec": bcol,
            "prior": np.ascontiguousarray(prior[b]),
        })
    try:
        res = run_bass_kernel_spmd(nc, in_maps, list(range(B)), trace=trace)
    except ModuleNotFoundError:
        res = run_bass_kernel_spmd(nc, in_maps, list(range(B)), trace=False)
    c = np.stack([res.results[i]["c_out"] for i in range(B)])
    na = np.stack([res.results[i]["na_out"] for i in range(B)])
    return (c, na), res


def kernel(**inputs):
    (c, na), _ = run(inputs)
    return (c, na)


# revision 11
# speedup vs baseline: 1.7298x; 1.1549x over previous
"""ConstituentAttention Trainium2 kernel.

Math (derived from the reference):
  - score is masked to the super/sub-diagonal only, so the row softmax is a
    2-element softmax: a_u[i] = sigmoid((s_u[i]-s_l[i])/E), a_l = 1-a_u,
    where s_u[i] = q_i.k_{i+1}, s_l[i] = q_i.k_{i-1}.
  - neighbor_attn = prior + (1-prior)*g where g == sqrt(1e-9) =: C0 everywhere
    except g[i,i+1] = g[i+1,i] = sqrt(a_u[i]*a_l[i+1] + 1e-9) =: g_u[i].
  - log-space prefix products collapse to c_attn[i,j] = exp(-|U[j]-U[i]|) for
    i != j, where U = exclusive prefix sum of u_i = log(na[i,i+1] + 1e-9);
    diagonal of c_attn = na[i,i].

Sharding: data-parallel over batch, one batch element per NeuronCore (B=8).

Layout notes: i = r*128 + p (partition p fast, block r = 0..7 slow), so the
per-index arrays live as [128, 8] SBUF tiles.  s_u/s_l are extracted from
[1, S]-ish linear staging rows with THREE free-dim offsets (i-1, i, i+1)
stacked as [128, 24] tiles, which turns every partition-shift the algorithm
needs into a free-dim offset.
"""

import numpy as np

import concourse.bass as bass
import concourse.tile as tile
from concourse import mybir
from concourse.bass_utils import run_bass_kernel_spmd

S, B, E, P = 1024, 8, 512, 64
P2 = 2 * P
NB = S // 128
C0 = float(np.sqrt(1e-9))
NEG = -1e30
F32 = mybir.dt.float32
F32R = mybir.dt.float32r
AF = mybir.ActivationFunctionType
ALU = mybir.AluOpType

_CACHE = {}


def _ap(handle_or_ap, offset, dims):
    a0 = handle_or_ap[:] if not isinstance(handle_or_ap, bass.AP) else handle_or_ap
    return bass.AP(tensor=a0.tensor, offset=offset, ap=[list(d) for d in dims])


def _r(ap):
    return ap.bitcast(F32R)


def _split_multi_waits(nc):
    """This toolchain's walrus accepts at most ONE embedded on_wait per
    instruction; hoist extras into standalone EventSemaphore waits just
    before the instruction on the same engine."""
    n = 0
    for bb in nc.main_func.blocks:
        new = []
        for ins in bb.instructions:
            si = ins.sync_info
            if si is not None and si.on_wait and len(si.on_wait) > 1:
                for w in si.on_wait[:-1]:
                    n += 1
                    wi = mybir.InstEventSemaphore(
                        name=f"I-waitsplit-{n}",
                        opcode="EventSemaphore",
                        engine=ins.engine,
                        sync_info=mybir.SyncInfo(on_wait=[w], on_update=[]),
                    )
                    try:
                        nc.register_instruction(wi)
                    except Exception:
                        pass
                    new.append(wi)
                si.on_wait = si.on_wait[-1:]
            new.append(ins)
        try:
            bb.instructions[:] = new
        except TypeError:
            bb.instructions = new
    return n


def build_nc():
    nc = bass.Bass()

    xT = nc.dram_tensor("xT", [E, S], F32, kind="ExternalInput")
    wT = nc.dram_tensor("wT", [E, P2], F32, kind="ExternalInput")
    bvec = nc.dram_tensor("bvec", [P2, 1], F32, kind="ExternalInput")
    prior = nc.dram_tensor("prior", [S, S], F32, kind="ExternalInput")
    na_out = nc.dram_tensor("na_out", [S, S], F32, kind="ExternalOutput")
    c_out = nc.dram_tensor("c_out", [S, S], F32, kind="ExternalOutput")

    # Window masks are [128,130]: for row-block r the band lives in absolute
    # cols [r*128-1, r*128+129); with window origin w0 = r*128-1 the super-diag
    # sits at rel col p+2, diag at p+1, sub-diag at p, independent of r.
    p_i = np.arange(128)[:, None]
    c_i = np.arange(130)[None, :]
    mu_h = nc.inline_tensor((c_i == p_i + 2).astype(np.float32), "mask_u")
    ml_h = nc.inline_tensor((c_i == p_i).astype(np.float32), "mask_l")
    md_h = nc.inline_tensor((c_i == p_i + 1).astype(np.float32), "mask_d")
    m1d_h = nc.inline_tensor((c_i != p_i + 1).astype(np.float32), "mask_1md")
    # lhsT for within-block inclusive cumsum over partitions: out = triu.T @ u
    triu_h = nc.inline_tensor(
        np.triu(np.ones((128, 128), np.float32)), "triu_ones"
    )
    ones_col_h = nc.inline_tensor(np.ones((128, 1), np.float32), "ones_col")
    ones_row_h = nc.inline_tensor(np.ones((1, 128), np.float32), "ones_row")

    with tile.TileContext(nc) as tc:
        with (
            tc.tile_pool(name="setup", bufs=1) as setup,
            tc.tile_pool(name="blk", bufs=3) as blk,
            tc.tile_pool(name="mm", bufs=2, space="PSUM") as mm,
            tc.tile_pool(name="mm1", bufs=2, space="PSUM") as mm1,
            tc.tile_pool(name="ps_small", bufs=2, space="PSUM") as ps_small,
            tc.tile_pool(name="psrep", bufs=1, space="PSUM") as psrep,
        ):
            # -------- critical-path loads first: wT then xT (chunked) -------
            wT_t = setup.tile([128, 4, P2], F32)
            nc.sync.dma_start(
                out=wT_t,
                in_=_ap(wT, 0, [[P2, 128], [128 * P2, 4], [1, P2]]))
            bias_t = setup.tile([128, 1], F32)
            nc.sync.dma_start(out=bias_t, in_=bvec[:])
            xT_t = setup.tile([128, 4, S], F32)
            for c in range(4):
                nc.sync.dma_start(
                    out=xT_t[:, c, :],
                    in_=_ap(xT, c * 128 * S, [[S, 128], [1, S]]))

            # constants via the Act-engine queue (keeps SP free for bulk)
            mu_t = setup.tile([128, 130], F32)
            nc.scalar.dma_start(out=mu_t, in_=mu_h[:])
            ml_t = setup.tile([128, 130], F32)
            nc.scalar.dma_start(out=ml_t, in_=ml_h[:])
            md_t = setup.tile([128, 130], F32)
            nc.scalar.dma_start(out=md_t, in_=md_h[:])
            m1d_t = setup.tile([128, 130], F32)
            nc.scalar.dma_start(out=m1d_t, in_=m1d_h[:])
            triu_t = setup.tile([128, 128], F32)
            nc.scalar.dma_start(out=triu_t, in_=triu_h[:])
            ones_col = setup.tile([128, 1], F32)
            nc.scalar.dma_start(out=ones_col, in_=ones_col_h[:])
            ones_row = setup.tile([1, 128], F32)
            nc.scalar.dma_start(out=ones_row, in_=ones_row_h[:])

            # preload the Sigmoid activation table during the idle head
            eps_t = setup.tile([128, 1], F32)
            nc.vector.memset(eps_t, 1e-9)
            warm_t = setup.tile([1, 1], F32)
            nc.scalar.activation(warm_t, eps_t[0:1, 0:1], AF.Sigmoid)

            # ------- prior band gathers (early; feed U chain + c diag) ------
            pr_u = setup.tile([128, NB], F32)            # prior[i, i+1]
            nc.gpsimd.dma_start(
                out=pr_u[:, 0:7],
                in_=_ap(prior, 1, [[S + 1, 128], [128 * (S + 1), 7]]))
            nc.gpsimd.dma_start(
                out=pr_u[0:127, 7:8],
                in_=_ap(prior, 896 * (S + 1) + 1, [[S + 1, 127], [1, 1]]))
            nc.vector.memset(pr_u[127:128, 7:8], 0.0)    # i=1023: no (i,i+1)
            pr_d = setup.tile([128, NB], F32)            # prior[i, i]
            nc.gpsimd.dma_start(
                out=pr_d, in_=_ap(prior, 0, [[S + 1, 128], [128 * (S + 1), 8]]))

            # ---------------- qT/kT = (x @ W.T).T halves  [64, S] ----------
            # fp32r matmuls: out free 512 >= 256 -> 1 cycle/row.
            qT_t = setup.tile([64, S], F32)
            kT_t = setup.tile([64, S], F32)
            for j in range(2):
                for half, dest_t in enumerate((qT_t, kT_t)):
                    ps = mm.tile([64, 512], F32, tag="mmbig")
                    for c in range(4):
                        nc.tensor.matmul(
                            ps[:],
                            lhsT=_r(wT_t[:, c, half * 64:(half + 1) * 64]),
                            rhs=_r(xT_t[:, c, j * 512:(j + 1) * 512]),
                            start=(c == 0),
                            stop=(c == 3),
                        )
                    if half == 0:
                        nc.vector.tensor_scalar_add(
                            dest_t[:, j * 512:(j + 1) * 512], ps,
                            bias_t[0:64, 0:1])
                    else:
                        nc.scalar.activation(
                            dest_t[:, j * 512:(j + 1) * 512], ps, AF.Identity,
                            bias=bias_t[64:128, 0:1])

            # adjacent-pair products (split at col 511 so the first half of
            # the band chain overlaps the j=1 matmuls); reduce over P=64
            # partitions via ones-matmul into [1,*] PSUM, then stage rows:
            #   su_stage[k] = s_u[k-1] = tu[k-1]  (s_u[1023] = -inf)
            #   sl_stage[k] = s_l[k-1] = tl[k-2]  (s_l[0] = -inf)
            su_st = setup.tile([1, 1026], F32)
            nc.vector.memset(su_st[:, 0:1], NEG)
            nc.vector.memset(su_st[:, 1024:1026], NEG)
            sl_st = setup.tile([1, 1026], F32)
            nc.vector.memset(sl_st[:, 0:2], NEG)
            nc.vector.memset(sl_st[:, 1025:1026], NEG)
            tu_t = setup.tile([64, S - 1], F32)
            tl_t = setup.tile([64, S - 1], F32)
            for lo, hi in ((0, 511), (511, 1023)):
                w = hi - lo
                nc.vector.tensor_mul(tu_t[:, lo:hi], qT_t[:, lo:hi],
                                     kT_t[:, lo + 1:hi + 1])
                nc.vector.tensor_mul(tl_t[:, lo:hi], qT_t[:, lo + 1:hi + 1],
                                     kT_t[:, lo:hi])
                for src_t, st_t, off, eng in ((tu_t, su_st, 1, nc.vector),
                                              (tl_t, sl_st, 2, nc.scalar)):
                    ps1 = mm1.tile([1, 512], F32, tag="ones")
                    nc.tensor.matmul(ps1[0:1, 0:w], lhsT=_r(ones_col[0:64, :]),
                                     rhs=_r(src_t[:, lo:hi]),
                                     start=True, stop=True)
                    if eng is nc.vector:
                        nc.vector.tensor_copy(st_t[:, off + lo:off + hi],
                                              ps1[0:1, 0:w])
                    else:
                        nc.scalar.activation(st_t[:, off + lo:off + hi],
                                             ps1[0:1, 0:w], AF.Copy)

            # [128, 24] stacks: col groups g=0,1,2 hold index offsets i-1,i,i+1
            s_uu = setup.tile([128, 3, NB], F32)
            s_ll = setup.tile([128, 3, NB], F32)
            for g in range(3):
                nc.scalar.dma_start(
                    out=s_uu[:, g, :],
                    in_=_ap(su_st[:], g, [[1, 128], [128, NB]]))
                nc.gpsimd.dma_start(
                    out=s_ll[:, g, :],
                    in_=_ap(sl_st[:], g, [[1, 128], [128, NB]]))

            # 2-element softmax via sigmoid on all 3 offset groups at once
            diff_t = setup.tile([128, 3, NB], F32)
            nc.vector.tensor_sub(diff_t, s_uu, s_ll)
            a_u = setup.tile([128, 3, NB], F32)
            nc.scalar.activation(a_u, diff_t, AF.Sigmoid, scale=1.0 / E)
            a_l = setup.tile([128, 3, NB], F32)
            nc.scalar.activation(a_l, diff_t, AF.Sigmoid, scale=-1.0 / E)

            # g_l[i] = g_u[i-1] = sqrt(a_u[i-1]*a_l[i] + eps)  (cols 0:8)
            # g_u[i]            = sqrt(a_u[i]*a_l[i+1] + eps)  (cols 8:16)
            gq_t = setup.tile([128, 2, NB], F32)
            nc.vector.tensor_mul(gq_t, _ap(a_u[:], 0, [[24, 128], [8, 2], [1, NB]]),
                                 _ap(a_l[:], 8, [[24, 128], [8, 2], [1, NB]]))
            g_t = setup.tile([128, 2, NB], F32)
            nc.vector.tensor_scalar(g_t, gq_t, 1e-9, 0.5,
                                    op0=ALU.add, op1=ALU.pow)
            g_l = g_t[:, 0, :]
            g_u = g_t[:, 1, :]

            # gu_c/gl_c = g - C0 for the banded block-loop update
            gc_t = setup.tile([128, 2, NB], F32)
            nc.vector.tensor_scalar_sub(gc_t, g_t, C0)
            gl_c = gc_t[:, 0, :]
            gu_c = gc_t[:, 1, :]

            # na[i,i+1] = g_u + pr_u*(1-g_u);  u = ln(na + eps)
            omg_t = setup.tile([128, NB], F32)
            nc.vector.tensor_scalar(omg_t, g_u, -1.0, 1.0, op0=ALU.mult,
                                    op1=ALU.add)
            t_tmp = setup.tile([128, NB], F32)
            nc.vector.tensor_mul(t_tmp, pr_u, omg_t)
            na_bu = setup.tile([128, NB], F32)
            nc.vector.tensor_add(na_bu, t_tmp, g_u)
            u_t = setup.tile([128, NB], F32)
            nc.scalar.activation(u_t, na_bu, AF.Ln, bias=eps_t[:, 0:1])
            nd_t = setup.tile([128, NB], F32)            # na[i, i]
            nc.scalar.activation(nd_t, pr_d, AF.Copy, bias=C0, scale=1.0 - C0)

            # ---- U = exclusive prefix sum of u (no DRAM round trips) ----
            inc_ps = ps_small.tile([128, NB], F32, tag="tiny")
            nc.tensor.matmul(inc_ps, lhsT=triu_t, rhs=u_t, start=True, stop=True)
            exc_t = setup.tile([128, NB], F32)
            nc.vector.tensor_sub(exc_t, inc_ps, u_t)

            cs_ps = ps_small.tile([1, NB], F32, tag="tiny")   # per-block sums
            nc.tensor.matmul(cs_ps, lhsT=ones_col, rhs=u_t, start=True, stop=True)
            bp_t = setup.tile([1, NB], F32)
            nc.vector.memset(bp_t[:, 0:1], 0.0)
            nc.vector.tensor_copy(bp_t[:, 1:8], cs_ps[0:1, 0:7])
            zer_t = setup.tile([1, NB], F32)
            nc.vector.memset(zer_t, 0.0)
            bpx_t = setup.tile([1, NB], F32)             # exclusive block prefix
            nc.vector.tensor_tensor_scan(bpx_t, bp_t, zer_t, 0.0,
                                         op0=ALU.add, op1=ALU.add)
            bpr_ps = ps_small.tile([128, NB], F32, tag="tiny")
            nc.tensor.matmul(bpr_ps, lhsT=ones_row, rhs=bpx_t, start=True,
                             stop=True)
            U_t = setup.tile([128, NB], F32)
            nc.vector.tensor_add(U_t, exc_t, bpr_ps)

            # U_rep[p, j] = U[j] via SBUF reshape DMA + ones broadcast matmul
            U_lin = setup.tile([1, S], F32)
            nc.scalar.dma_start(out=_ap(U_lin[:], 0, [[1, 128], [128, NB]]),
                                in_=U_t)
            Ur_ps = psrep.tile([128, S], F32, tag="urep")
            for lo in (0, 512):
                nc.tensor.matmul(Ur_ps[:, lo:lo + 512], lhsT=_r(ones_row),
                                 rhs=_r(U_lin[0:1, lo:lo + 512]), start=True,
                                 stop=True)

            # ---------------- main per-row-block loop ----------------
            for r in range(NB):
                w0 = r * 128 - 1
                wlo = max(w0, 0)
                whi = min(w0 + 130, S)
                wd = whi - wlo
                mo = wlo - w0

                pr_t = blk.tile([128, S], F32, tag="pr")
                nc.sync.dma_start(out=pr_t, in_=prior[r * 128:(r + 1) * 128, :])

                # full na row = affine(prior): no band dependency, write it out
                # immediately; the tiny band window is patched by a second DMA.
                na_t = blk.tile([128, S], F32, tag="na")
                nc.scalar.activation(na_t, pr_t, AF.Copy, bias=C0, scale=1.0 - C0)
                nc.sync.dma_start(out=na_out[r * 128:(r + 1) * 128, :], in_=na_t)

                # band window: g = C0 + M_u*(g_u-C0) + M_l*(g_l-C0)
                gwin = blk.tile([128, 130], F32, tag="gwin")
                nc.vector.tensor_scalar(gwin[:, :wd], mu_t[:, mo:mo + wd],
                                        gu_c[:, r:r + 1], C0,
                                        op0=ALU.mult, op1=ALU.add)
                t2w = blk.tile([128, 130], F32, tag="t2w")
                nc.vector.tensor_scalar(t2w[:, :wd], ml_t[:, mo:mo + wd],
                                        gl_c[:, r:r + 1], None, op0=ALU.mult)
                gw2 = blk.tile([128, 130], F32, tag="gw2")
                nc.vector.tensor_add(gw2[:, :wd], gwin[:, :wd], t2w[:, :wd])
                # na_win = g + prior*(1-g) = g + prior - prior*g
                t3w = blk.tile([128, 130], F32, tag="t3w")
                nc.vector.tensor_mul(t3w[:, :wd], pr_t[:, wlo:whi], gw2[:, :wd])
                t4w = blk.tile([128, 130], F32, tag="t4w")
                nc.vector.tensor_sub(t4w[:, :wd], pr_t[:, wlo:whi], t3w[:, :wd])
                naw = blk.tile([128, 130], F32, tag="naw")
                nc.vector.tensor_add(naw[:, :wd], t4w[:, :wd], gw2[:, :wd])
                nc.sync.dma_start(
                    out=_ap(na_out, r * 128 * S + wlo, [[S, 128], [1, wd]]),
                    in_=naw[:, :wd])

                # c_attn block: exp(-|U[j] - U[i]|), diag <- na[i,i]
                cd_t = blk.tile([128, S], F32, tag="cd")
                nc.vector.tensor_scalar(cd_t, Ur_ps, U_t[:, r:r + 1], 0.0,
                                        op0=ALU.subtract, op1=ALU.abs_max)
                c2_t = blk.tile([128, S], F32, tag="c2")
                nc.scalar.activation(c2_t, cd_t, AF.Exp, scale=-1.0)
                t5w = blk.tile([128, 130], F32, tag="t5w")
                nc.vector.tensor_scalar(t5w[:, :wd], md_t[:, mo:mo + wd],
                                        nd_t[:, r:r + 1], None, op0=ALU.mult)
                t6w = blk.tile([128, 130], F32, tag="t6w")
                nc.vector.tensor_mul(t6w[:, :wd], c2_t[:, wlo:whi],
                                     m1d_t[:, mo:mo + wd])
                nc.vector.tensor_add(c2_t[:, wlo:whi], t5w[:, :wd], t6w[:, :wd])

                nc.sync.dma_start(out=c_out[r * 128:(r + 1) * 128, :], in_=c2_t)

    _split_multi_waits(nc)
    return nc


def _get_nc():
    if "nc" not in _CACHE:
        _CACHE["nc"] = build_nc()
    return _CACHE["nc"]


def run(inputs, trace=False):
    nc = _get_nc()
    context = np.asarray(inputs["context"], np.float32)
    prior = np.asarray(inputs["prior"], np.float32)
    w = np.asarray(inputs["proj_weight"], np.float32)
    bias = np.asarray(inputs["proj_bias"], np.float32)

    wT = np.ascontiguousarray(w.T)                     # [E, 2P]
    bcol = np.ascontiguousarray(bias.reshape(P2, 1))
    in_maps = []
    for b in range(B):
        in_maps.append({
            "xT": np.ascontiguousarray(context[:, b, :].T),   # [E, S]
            "wT": wT,
            "bvec": bcol,
            "prior": np.ascontiguousarray(prior[b]),
        })
    try:
        res = run_bass_kernel_spmd(nc, in_maps, list(range(B)), trace=trace)
    except ModuleNotFoundError:
        res = run_bass_kernel_spmd(nc, in_maps, list(range(B)), trace=False)
    c = np.stack([res.results[i]["c_out"] for i in range(B)])
    na = np.stack([res.results[i]["na_out"] for i in range(B)])
    return (c, na), res


def kernel(**inputs):
    (c, na), _ = run(inputs)
    return (c, na)
